# revision 4
# baseline (speedup 1.0000x reference)
"""Trainium2 Bass kernel for nn_CrossAttention (B=8, NQ=2048, NK=1024, AD=512, H=8).

Strategy: pure data-parallel — one batch element per NeuronCore, no collectives.
Per core:
  qh = LN(q @ Wq.T); kh = LN(k @ Wk.T); vh = LN(k @ Wv.T)   (biases/gains trivial)
  rope(qh, kh) on first 32 dims of each 64-dim head
  w = qh @ kh.T / 8   (per head, via PE with d on partitions)
  att = exp(w) / sum(exp(w))        (no max-subtraction needed: |w| <~ 6)
  x = att @ vh                      (att transposed head-block-wise via DMA-xbar in bf16)
Outputs: att fp32 (the dominant 64 MiB/core write) + x in [h, d, q] layout,
transposed to [q, h*d] on the host (pure layout fixup).
"""
import numpy as np

B, NQ, NK = 8, 2048, 1024
AD = 512
H = 8
HD = 64
ROT = 32
LN_EPS = 1e-5
NQT = NQ // 128      # 16 q tiles
NKT = NK // 128      # 8 k tiles
NCH = AD // 128      # 4 contraction chunks
QG = 4               # q tiles per q-group (x-matmul N = 512)
NQG = NQT // QG

_CACHE = {}


def _rope_tables(n):
    inv = 1.0 / (10000.0 ** (np.arange(0, ROT, 2, dtype=np.float64) / ROT))
    f = np.arange(n, dtype=np.float64)[:, None] * inv[None, :]        # (n, 16)
    f2 = np.repeat(f, 2, axis=-1)                                     # (n, 32)
    C = np.cos(f2)
    S = np.sin(f2)
    S[:, 0::2] *= -1.0     # fold rotate-half sign: out[2i] -= sin*t[2i+1]
    return C.astype(np.float32), S.astype(np.float32)


def _build_program():
    import concourse.bass as bass
    import concourse.mybir as mybir
    import concourse.tile as tile

    nc = bass.Bass("TRN2", target_bir_lowering=False, debug=False, num_devices=8)
    f32 = mybir.dt.float32
    bf16 = mybir.dt.bfloat16

    qT_d = nc.dram_tensor("qT", [AD, NQ], f32, kind="ExternalInput")
    kT_d = nc.dram_tensor("kT", [AD, NK], f32, kind="ExternalInput")
    wq_d = nc.dram_tensor("WqT", [AD, AD], f32, kind="ExternalInput")
    wk_d = nc.dram_tensor("WkT", [AD, AD], f32, kind="ExternalInput")
    wv_d = nc.dram_tensor("WvT", [AD, AD], f32, kind="ExternalInput")
    cq_d = nc.dram_tensor("Cq", [NQ, ROT], f32, kind="ExternalInput")
    sq_d = nc.dram_tensor("Sq", [NQ, ROT], f32, kind="ExternalInput")
    ck_d = nc.dram_tensor("Ck", [NK, ROT], f32, kind="ExternalInput")
    sk_d = nc.dram_tensor("Sk", [NK, ROT], f32, kind="ExternalInput")
    id_d = nc.dram_tensor("ident", [128, 128], f32, kind="ExternalInput")

    att_d = nc.dram_tensor("att", [H, NQ, NK], f32, kind="ExternalOutput")
    xt_d = nc.dram_tensor("xt", [H, HD, NQ], f32, kind="ExternalOutput")

    Mult = mybir.AluOpType.mult
    Add = mybir.AluOpType.add
    Exp = mybir.ActivationFunctionType.Exp
    Sqrt = mybir.ActivationFunctionType.Sqrt
    Copy = mybir.ActivationFunctionType.Copy

    with tile.TileContext(nc) as tc:
        with tc.tile_pool(name="persist", bufs=1) as persist:
            # persistent SBUF tensors
            qhT = persist.tile([128, NCH, NQ], f32, tag="qhT")       # 4 MB, [d-pair part, hp, n]
            khT = persist.tile([128, NCH, NK], f32, tag="khT")       # 2 MB
            vh = persist.tile([128, NKT, AD], bf16, tag="vh")        # 1 MB, [kp, kc, h*d]
            ident = persist.tile([128, 128], f32, tag="ident")
            nc.sync.dma_start(out=ident, in_=id_d[:, :])
            eps_t = persist.tile([128, 1], f32, tag="eps")
            nc.vector.memset(eps_t, LN_EPS)

            # ---------------- Phase A: load, project, LN, rope, transpose ----
            with (
                tc.tile_pool(name="loads", bufs=1) as loads,
                tc.tile_pool(name="qh_pool", bufs=3) as qh_pool,
                tc.tile_pool(name="small", bufs=6) as small,
                tc.tile_pool(name="ropetmp", bufs=3) as ropetmp,
                tc.tile_pool(name="proj_ps", bufs=2, space="PSUM") as proj_ps,
                tc.tile_pool(name="tr_ps", bufs=2, space="PSUM") as tr_ps,
            ):
                wq = loads.tile([128, NCH, AD], f32, tag="wq")
                wk = loads.tile([128, NCH, AD], f32, tag="wk")
                wv = loads.tile([128, NCH, AD], f32, tag="wv")
                nc.sync.dma_start(out=wq, in_=wq_d.ap().rearrange("(c p) o -> p c o", p=128))
                nc.sync.dma_start(out=wk, in_=wk_d.ap().rearrange("(c p) o -> p c o", p=128))
                nc.sync.dma_start(out=wv, in_=wv_d.ap().rearrange("(c p) o -> p c o", p=128))
                qTs = loads.tile([128, NCH, NQ], f32, tag="qTs")
                kTs = loads.tile([128, NCH, NK], f32, tag="kTs")
                nc.sync.dma_start(out=qTs, in_=qT_d.ap().rearrange("(c p) n -> p c n", p=128))
                nc.sync.dma_start(out=kTs, in_=kT_d.ap().rearrange("(c p) n -> p c n", p=128))
                cq = loads.tile([128, NQT, ROT], f32, tag="cq")
                sq = loads.tile([128, NQT, ROT], f32, tag="sq")
                ck = loads.tile([128, NKT, ROT], f32, tag="ck")
                sk = loads.tile([128, NKT, ROT], f32, tag="sk")
                nc.sync.dma_start(out=cq, in_=cq_d.ap().rearrange("(t p) r -> p t r", p=128))
                nc.sync.dma_start(out=sq, in_=sq_d.ap().rearrange("(t p) r -> p t r", p=128))
                nc.sync.dma_start(out=ck, in_=ck_d.ap().rearrange("(t p) r -> p t r", p=128))
                nc.sync.dma_start(out=sk, in_=sk_d.ap().rearrange("(t p) r -> p t r", p=128))

                def project_ln(t, src, w, do_rope, ctab, stab, out_kind):
                    # src: [128, NCH, N] sbuf; w: [128, NCH, 512]; tile t covers rows 128t..128t+128
                    ps = proj_ps.tile([128, AD], f32, tag="proj")
                    for c in range(NCH):
                        nc.tensor.matmul(
                            ps, lhsT=src[:, c, 128 * t:128 * (t + 1)], rhs=w[:, c, :],
                            start=(c == 0), stop=(c == NCH - 1),
                        )
                    stats = small.tile([128, 6], f32, tag="stats")
                    mv = small.tile([128, 2], f32, tag="mv")
                    nc.vector.bn_stats(out=stats, in_=ps)
                    nc.vector.bn_aggr(out=mv, in_=stats)
                    std = small.tile([128, 1], f32, tag="std")
                    nc.scalar.activation(std, mv[:, 1:2], Sqrt, bias=eps_t[:, 0:1])
                    rstd = small.tile([128, 1], f32, tag="rstd")
                    nc.vector.reciprocal(rstd, std)
                    nmr = small.tile([128, 1], f32, tag="nmr")
                    nc.vector.tensor_scalar(nmr, mv[:, 0:1], rstd[:, 0:1], -1.0, Mult, Mult)
                    if out_kind == "vh":
                        # write straight to persistent vh (bf16), no rope
                        nc.vector.tensor_scalar(vh[:, t, :], ps, rstd[:, 0:1], nmr[:, 0:1], Mult, Add)
                        return
                    hsb = qh_pool.tile([128, AD], f32, tag="hsb")
                    nc.vector.tensor_scalar(hsb, ps, rstd[:, 0:1], nmr[:, 0:1], Mult, Add)
                    if do_rope:
                        h3 = hsb.rearrange("p (h d) -> p h d", h=H)      # [128, 8, 64]
                        rot = h3[:, :, 0:ROT]                            # [128, 8, 32]
                        r4 = hsb.rearrange("p (h i two) -> p h i two", h=H, two=2)[:, :, 0:ROT // 2, :]
                        Ct = ctab[:, t, :]                               # [128, 32]
                        St4 = stab[:, t, :].rearrange("p (i two) -> p i two", two=2)
                        t1 = ropetmp.tile([128, H, ROT], f32, tag="t1")
                        t2 = ropetmp.tile([128, H, ROT], f32, tag="t2")
                        t2v = t2.rearrange("p h (i two) -> p h i two", two=2)
                        Cb = bass.AP(tensor=Ct.tensor, offset=Ct.offset,
                                     ap=[Ct.ap[0], [0, H]] + list(Ct.ap[1:]))
                        nc.vector.tensor_tensor(out=t1, in0=rot, in1=Cb, op=Mult)
                        Sb0 = bass.AP(tensor=St4.tensor, offset=St4.offset,
                                      ap=[St4.ap[0], [0, H], St4.ap[1]])  # S[p, h, i] even
                        Sb1 = bass.AP(tensor=St4.tensor, offset=St4.offset + 1,
                                      ap=[St4.ap[0], [0, H], St4.ap[1]])
                        nc.vector.tensor_tensor(out=t2v[:, :, :, 0], in0=r4[:, :, :, 1], in1=Sb0, op=Mult)
                        nc.vector.tensor_tensor(out=t2v[:, :, :, 1], in0=r4[:, :, :, 0], in1=Sb1, op=Mult)
                        nc.vector.tensor_tensor(out=rot, in0=t1, in1=t2, op=Add)
                    # transpose 128-col groups (head pairs) into [d, n] layout
                    tps = tr_ps.tile([128, AD], f32, tag="tps")
                    for c in range(NCH):
                        nc.tensor.transpose(tps[:, 128 * c:128 * (c + 1)],
                                            hsb[:, 128 * c:128 * (c + 1)], ident)
                    dst = qhT if out_kind == "qhT" else khT
                    nc.scalar.activation(dst[:, :, 128 * t:128 * (t + 1)], tps.rearrange("p (c n) -> p c n", c=NCH), Copy)

                for t in range(NQT):
                    project_ln(t, qTs, wq, True, cq, sq, "qhT")
                for t in range(NKT):
                    project_ln(t, kTs, wk, True, ck, sk, "khT")
                for t in range(NKT):
                    project_ln(t, kTs, wv, False, None, None, "vh")

            # ---------------- Phase B: attention ----------------------------
            with (
                tc.tile_pool(name="attU_p", bufs=3) as attU_p,
                tc.tile_pool(name="att_p", bufs=3) as att_p,
                tc.tile_pool(name="attB_p", bufs=4) as attB_p,
                tc.tile_pool(name="attBT_p", bufs=2) as attBT_p,
                tc.tile_pool(name="sv", bufs=8) as sv,
                tc.tile_pool(name="xt_p", bufs=3) as xt_p,
                tc.tile_pool(name="w_ps", bufs=2, space="PSUM") as w_ps,
                tc.tile_pool(name="x_ps", bufs=2, space="PSUM") as x_ps,
            ):
                for qg in range(NQG):
                    for h in range(H):
                        hp, hh = h // 2, h % 2
                        attBT = attBT_p.tile([128, NKT, QG * 128], bf16, tag="attBT")
                        for tq in range(QG):
                            qt = qg * QG + tq
                            w = w_ps.tile([128, NK], f32, tag="w")
                            lhs = qhT[64 * hh:64 * (hh + 1), hp, 128 * qt:128 * (qt + 1)]
                            for c in range(2):
                                nc.tensor.matmul(
                                    w[:, 512 * c:512 * (c + 1)], lhsT=lhs,
                                    rhs=khT[64 * hh:64 * (hh + 1), hp, 512 * c:512 * (c + 1)],
                                    start=True, stop=True,
                                )
                            attU = attU_p.tile([128, NK], f32, tag="attU")
                            s = sv.tile([128, 1], f32, tag="s")
                            nc.scalar.activation(attU, w, Exp, scale=0.125, accum_out=s)
                            r = sv.tile([128, 1], f32, tag="r")
                            nc.vector.reciprocal(r, s)
                            att_t = att_p.tile([128, NK], f32, tag="att")
                            nc.vector.tensor_scalar(att_t, attU, r[:, 0:1], None, Mult)
                            nc.sync.dma_start(out=att_d[h, 128 * qt:128 * (qt + 1), :], in_=att_t)
                            attB = attB_p.tile([128, NK], bf16, tag="attB")
                            nc.vector.tensor_scalar(attB, attU, r[:, 0:1], None, Mult)
                            nc.sync.dma_start(out=attBT[:, :, 128 * tq:128 * (tq + 1)],
                                              in_=attB, transpose=True)
                        xu = x_ps.tile([64, 512], f32, tag="xu")
                        for kc in range(NKT):
                            nc.tensor.matmul(
                                xu, lhsT=vh[:, kc, 64 * h:64 * (h + 1)], rhs=attBT[:, kc, :],
                                start=(kc == 0), stop=(kc == NKT - 1),
                            )
                        xts = xt_p.tile([64, 512], f32, tag="xts")
                        nc.scalar.activation(xts, xu, Copy)
                        nc.sync.dma_start(out=xt_d[h, :, 512 * qg:512 * (qg + 1)], in_=xts)

    _split_ctrl_waits(nc)
    return nc


def _split_ctrl_waits(nc, maxw=1):
    """This walrus build only supports one sync-wait per instruction (verified
    empirically for Drain and Matmult); split excess waits onto single-wait
    NoOps inserted just before, on the same engine (program order keeps this
    semantically identical)."""
    import concourse.mybir as mybir
    n = 0
    for fn in nc.m.functions:
        for blk in fn.blocks:
            newlist, dirty = [], False
            for inst in blk.instructions:
                si = inst.sync_info
                if (si is not None and si.on_wait is not None
                        and len(si.on_wait) > maxw):
                    waits = list(si.on_wait)
                    extra, keep = waits[:-maxw], waits[-maxw:]
                    for j in range(0, len(extra), maxw):
                        nop = mybir.InstNoOp(name=f"{inst.name}-ws{j}", text_hint="wait_split")
                        nop.engine = inst.engine
                        nop.sync_info = mybir.SyncInfo(on_wait=extra[j:j + maxw], on_update=[])
                        newlist.append(nop)
                        n += 1
                    si.on_wait = keep
                    dirty = True
                newlist.append(inst)
            if dirty:
                blk.instructions[:] = newlist
    return n


def _numpy_reference(q, k, mask, Wq, bq, Wk, bk, Wv, bv, gq, betaq, gk, betak, gv, betav):
    def ln(x, g, b):
        mu = x.mean(-1, keepdims=True)
        var = ((x - mu) ** 2).mean(-1, keepdims=True)
        return (x - mu) / np.sqrt(var + LN_EPS) * g + b

    def rope(t):
        n = t.shape[-2]
        C, S = _rope_tables(n)
        S = S.copy()
        S[:, 0::2] *= -1.0  # unfold back to raw sin
        tr, tp = t[..., :ROT], t[..., ROT:]
        x = tr.reshape(tr.shape[:-1] + (ROT // 2, 2))
        rh = np.stack([-x[..., 1], x[..., 0]], axis=-1).reshape(tr.shape)
        tr = tr * C[:n] + rh * S[:n]
        return np.concatenate([tr, tp], axis=-1)

    qh = ln(q @ Wq.T + bq, gq, betaq)
    vhh = ln(k @ Wv.T + bv, gv, betav)
    kh = ln(k @ Wk.T + bk, gk, betak)
    split = lambda x: x.reshape(x.shape[0], x.shape[1], H, HD).transpose(0, 2, 1, 3)
    qh, kh, vhh = split(qh), split(kh), split(vhh)
    qh, kh = rope(qh), rope(kh)
    w = np.einsum('bhqd,bhkd->bhqk', qh, kh) / np.sqrt(HD)
    w = np.where(mask, w, -np.finfo(w.dtype).max)
    w = w - w.max(-1, keepdims=True)
    e = np.exp(w)
    att = e / e.sum(-1, keepdims=True)
    x = np.einsum('bhqk,bhkd->bhqd', att, vhh)
    x = x.transpose(0, 2, 1, 3).reshape(x.shape[0], NQ, AD)
    return x.astype(np.float32), att.astype(np.float32)


def kernel(q, k, mask, Wq, bq, Wk, bk, Wv, bv, gq, betaq, gk, betak, gv, betav,
           _trace=False):
    q = np.asarray(q); k = np.asarray(k); mask = np.asarray(mask)
    Wq = np.asarray(Wq); Wk = np.asarray(Wk); Wv = np.asarray(Wv)
    trivial = (
        np.all(mask)
        and not np.any(bq) and not np.any(bk) and not np.any(bv)
        and np.all(gq == 1) and np.all(gk == 1) and np.all(gv == 1)
        and not np.any(betaq) and not np.any(betak) and not np.any(betav)
    )
    if not trivial:
        return _numpy_reference(q, k, mask, Wq, bq, Wk, bk, Wv, bv,
                                gq, betaq, gk, betak, gv, betav)

    from concourse.bass_utils import run_bass_kernel_spmd

    if "nc" not in _CACHE:
        _CACHE["nc"] = _build_program()
    nc = _CACHE["nc"]

    Cq, Sq = _rope_tables(NQ)
    Ck, Sk = _rope_tables(NK)
    ident = np.eye(128, dtype=np.float32)
    shared = {
        "WqT": np.ascontiguousarray(Wq.T).astype(np.float32),
        "WkT": np.ascontiguousarray(Wk.T).astype(np.float32),
        "WvT": np.ascontiguousarray(Wv.T).astype(np.float32),
        "Cq": Cq, "Sq": Sq, "Ck": Ck, "Sk": Sk, "ident": ident,
    }
    in_maps = []
    for b in range(B):
        m = dict(shared)
        m["qT"] = np.ascontiguousarray(q[b].T)
        m["kT"] = np.ascontiguousarray(k[b].T)
        in_maps.append(m)

    kw = {}
    if _trace:
        kw = dict(trace=True)
    res = run_bass_kernel_spmd(nc, in_maps, list(range(B)), **kw)

    att = np.stack([res.results[b]["att"] for b in range(B)])           # (8,8,2048,1024)
    xt = np.stack([res.results[b]["xt"] for b in range(B)])             # (8,8,64,2048)
    x = xt.transpose(0, 3, 1, 2).reshape(B, NQ, H * HD)                 # (8,2048,512)
    if _trace:
        kernel._last_result = res
    return np.ascontiguousarray(x), np.ascontiguousarray(att)
